# revision 2
# baseline (speedup 1.0000x reference)
"""Trainium2 Bass kernel for nn_Model2_3925600109170 (gnn_message_passing).

Only the news->news GAT + MLP head + final row-gather affect the output
(the SAGE and news->topic GAT results are computed then deleted in the
reference), and the final gather keeps only the 1024 queried rows:

    out[q] depends only on edges with links_dst == news_indices[q]

so of the 1.6M edges only ~16k matter.  Host-side index work selects
those edges; all floating-point model math runs on device:

    g  = x_src.T @ [ws.T | ws.T@a_s]          per edge  -> [hs | es]
    ed = x_dst.T @ (wd.T@a_d)                 per edge
    w  = exp(leaky_relu(es + ed, 0.2))        (softmax shift skipped: |e|<~2,
                                               ratio is shift-invariant)
    num/den = one-hot matmul segment-sum of [w*hs | w] per dst slot
    h  = num / max(den, 1e-16)   (+gat bias folded into lin1 bias)
    out= relu(h @ W1.T + b1') @ W2.T + b2

Sharding: the 1024 query slots are split 128 per core; each core
processes only its slots' edges (~2k), fully independently.
"""

import numpy as np

N_NEWS = 100_000
D = 128
H = 64
Q = 1024
SLOTS = 128              # query slots per core
NCORES = 8

_CACHE = {}


def _bf16(a):
    import ml_dtypes
    return np.asarray(a, dtype=ml_dtypes.bfloat16)


def _ranges_concat(starts, ends):
    """Concatenate [starts[i], ends[i]) index ranges. (vectorized)"""
    cnt = ends - starts
    total = int(cnt.sum())
    if total == 0:
        return np.zeros(0, np.int64)
    step = np.ones(total, np.int64)
    step[0] = starts[0]
    cum = np.cumsum(cnt)[:-1]
    nz = cnt > 0
    # positions where a new range begins (skip empty ranges)
    firsts = np.flatnonzero(nz)
    pos = np.concatenate([[0], cum])[firsts]
    step[pos] = starts[firsts]
    step[pos[1:]] -= ends[firsts[:-1]] - 1
    return np.cumsum(step)


def _host_prep(x_news, ws, a_s, wd, a_d, b, w1, b1, w2, b2,
               links_src, links_dst, n_id, news_indices):
    f32 = np.float32
    src = np.asarray(links_src, np.int64)
    dst = np.asarray(links_dst, np.int64)
    local = np.searchsorted(np.asarray(n_id, np.int64),
                            np.asarray(news_indices, np.int64))

    mask = np.zeros(N_NEWS, bool)
    mask[local] = True
    em = mask[dst]
    fs, fd = src[em], dst[em]
    order = np.argsort(fd, kind="stable")
    fs, fd = fs[order], fd[order]
    starts = np.searchsorted(fd, local)
    ends = np.searchsorted(fd, local, side="right")
    cnt = ends - starts                                  # per slot degree

    percore = cnt.reshape(NCORES, SLOTS).sum(axis=1)
    NC = int(max(1, -(-int(percore.max()) // 128)))
    NC = ((NC + 1) // 2) * 2                             # even: fewer recompiles
    E_PAD = NC * 128

    xt16 = _bf16(np.ascontiguousarray(np.asarray(x_news, f32).T))   # [128, N]

    cbf = np.zeros((D, 66), f32)
    cbf[:, 0:64] = ws.T
    cbf[:, 64] = ws.T @ a_s
    cbf[:, 65] = wd.T @ a_d
    cbf = _bf16(cbf)

    cf32 = np.zeros((D, 98), f32)
    cf32[0:64, 0:64] = w1.T
    cf32[0:64, 64] = w1 @ b + b1
    cf32[0:64, 65:97] = w2.T
    cf32[0:32, 97] = b2

    in_maps = []
    for c in range(NCORES):
        s0, s1 = c * SLOTS, (c + 1) * SLOTS
        idx = _ranges_concat(starts[s0:s1], ends[s0:s1])
        n = idx.size
        e_src = fs[idx]
        e_slot = np.repeat(np.arange(SLOTS), cnt[s0:s1])
        dat = np.zeros((D, 3 * E_PAD), xt16.dtype)
        dat[:, 0:n] = xt16[:, e_src]
        dat[:, E_PAD:E_PAD + n] = xt16[:, local[s0 + e_slot]]
        oh = dat[:, 2 * E_PAD:].reshape(D, NC, 128)
        i = np.arange(n)
        oh[i % 128, i // 128, e_slot] = 1
        in_maps.append(dict(dat=dat, cbf=cbf, cf32=cf32))

    return in_maps, NC, local


def _build_program(NC):
    import concourse.bass as bass
    import concourse.bacc as bacc
    import concourse.mybir as mybir
    import concourse.tile as tile

    f32, bf16 = mybir.dt.float32, mybir.dt.bfloat16
    AO = mybir.AluOpType
    AF = mybir.ActivationFunctionType
    E_PAD = NC * 128
    GT = 7                                   # g chunks per PSUM bank
    NT = -(-NC // GT)                        # number of g psum tiles

    nc = bacc.Bacc("TRN2", target_bir_lowering=False, debug=False,
                   num_devices=NCORES)

    dat = nc.dram_tensor("dat", [D, 3 * E_PAD], bf16, kind="ExternalInput")
    cbf = nc.dram_tensor("cbf", [D, 66], bf16, kind="ExternalInput")
    cf32 = nc.dram_tensor("cf32", [D, 98], f32, kind="ExternalInput")
    outt = nc.dram_tensor("outt", [32, SLOTS], f32, kind="ExternalOutput")

    with tile.TileContext(nc) as tc:
        with (
            tc.tile_pool(name="const", bufs=1) as constp,
            tc.tile_pool(name="data", bufs=1) as datp,
            tc.tile_pool(name="wrk", bufs=1) as wrk,
            tc.tile_pool(name="gps", bufs=1, space="PSUM") as gpsp,
            tc.tile_pool(name="edps", bufs=1, space="PSUM") as edpsp,
            tc.tile_pool(name="aggps", bufs=1, space="PSUM") as aggpsp,
            tc.tile_pool(name="smps", bufs=2, space="PSUM") as smpsp,
        ):
            cbf_t = constp.tile([D, 66], bf16)
            nc.sync.dma_start(out=cbf_t[:], in_=cbf.ap())
            cf32_t = constp.tile([D, 98], f32)
            nc.sync.dma_start(out=cf32_t[:], in_=cf32.ap())
            ones_t = constp.tile([1, H], f32)
            nc.vector.memset(ones_t[:], 1.0)

            xe_t = datp.tile([D, E_PAD], bf16)
            xde_t = datp.tile([D, E_PAD], bf16)
            oh_t = datp.tile([D, E_PAD], bf16)
            NSL = 2                            # dma slices per tensor
            for t, base in ((xe_t, 0), (xde_t, E_PAD), (oh_t, 2 * E_PAD)):
                w = E_PAD // NSL
                for s in range(NSL):
                    nc.sync.dma_start(
                        out=t[:, s * w:(s + 1) * w],
                        in_=dat.ap()[:, base + s * w: base + (s + 1) * w])

            # g = [hs | es] per edge: chunks of 128 edges on partitions
            g_ps = [gpsp.tile([128, GT, 65], f32, space="PSUM", tag=f"g{t}")
                    for t in range(NT)]
            for c in range(NC):
                nc.tensor.matmul(out=g_ps[c // GT][:, c % GT, :],
                                 lhsT=xe_t[:, c * 128:(c + 1) * 128],
                                 rhs=cbf_t[:, 0:65], start=True, stop=True)
            # ed per edge
            ed_ps = edpsp.tile([128, NC], f32, space="PSUM")
            for c in range(NC):
                nc.tensor.matmul(out=ed_ps[:, c:c + 1],
                                 lhsT=xde_t[:, c * 128:(c + 1) * 128],
                                 rhs=cbf_t[:, 65:66], start=True, stop=True)

            # w = exp(leaky_relu(es + ed))
            l_t = wrk.tile([128, NC], f32)
            for t in range(NT):
                k = min(GT, NC - t * GT)
                nc.vector.tensor_tensor(
                    out=l_t[:, t * GT:t * GT + k],
                    in0=g_ps[t][:, 0:k, 64],
                    in1=ed_ps[:, t * GT:t * GT + k], op=AO.add)
            lr_t = wrk.tile([128, NC], f32)
            nc.scalar.activation(lr_t[:], l_t[:], AF.Lrelu, alpha=0.2)
            w_t = wrk.tile([128, NC], f32)
            nc.scalar.activation(w_t[:], lr_t[:], AF.Exp)

            # gw = [w*hs | w] in bf16 for the aggregation matmuls
            gw_t = wrk.tile([128, NC, 65], bf16)
            for t in range(NT):
                k = min(GT, NC - t * GT)
                nc.vector.tensor_tensor(
                    out=gw_t[:, t * GT:t * GT + k, 0:64],
                    in0=g_ps[t][:, 0:k, 0:64],
                    in1=w_t[:, t * GT:t * GT + k]
                        .rearrange("p n -> p n 1").to_broadcast([128, k, 64]),
                    op=AO.mult)
            nc.vector.tensor_copy(out=gw_t[:, :, 64], in_=w_t[:, 0:NC])

            # segment sum via one-hot matmul: agg[65, slot] = sum_e gw[e]*oh[e,slot]
            agg_ps = aggpsp.tile([65, SLOTS], f32, space="PSUM")
            for c in range(NC):
                nc.tensor.matmul(out=agg_ps[:],
                                 lhsT=gw_t[:, c, :],
                                 rhs=oh_t[:, c * 128:(c + 1) * 128],
                                 start=(c == 0), stop=(c == NC - 1))

            # h = num/den ; MLP head
            den_t = wrk.tile([1, SLOTS], f32)
            nc.vector.tensor_scalar_max(den_t[:], agg_ps[64:65, :], 1e-16)
            rec_t = wrk.tile([1, SLOTS], f32)
            nc.vector.reciprocal(rec_t[:], den_t[:])
            rbc_ps = smpsp.tile([H, SLOTS], f32, space="PSUM", tag="sm")
            nc.tensor.matmul(out=rbc_ps[:], lhsT=ones_t[:], rhs=rec_t[:],
                             start=True, stop=True)
            rbc_t = wrk.tile([H, SLOTS], f32)
            nc.vector.tensor_copy(out=rbc_t[:], in_=rbc_ps[:])
            ht_t = wrk.tile([H, SLOTS], f32)
            nc.vector.tensor_tensor(out=ht_t[:], in0=agg_ps[0:64, :],
                                    in1=rbc_t[:], op=AO.mult)
            mm1_ps = smpsp.tile([H, SLOTS], f32, space="PSUM", tag="sm")
            nc.tensor.matmul(out=mm1_ps[:], lhsT=cf32_t[0:64, 0:64],
                             rhs=ht_t[:], start=True, stop=True)
            x1_t = wrk.tile([H, SLOTS], f32)
            nc.scalar.activation(x1_t[:], mm1_ps[:], AF.Relu,
                                 bias=cf32_t[0:64, 64:65], scale=1.0)
            mm2_ps = smpsp.tile([32, SLOTS], f32, space="PSUM", tag="sm")
            nc.tensor.matmul(out=mm2_ps[:], lhsT=cf32_t[0:64, 65:97],
                             rhs=x1_t[:], start=True, stop=True)
            out_t = wrk.tile([32, SLOTS], f32)
            nc.vector.tensor_scalar(out=out_t[:], in0=mm2_ps[:],
                                    scalar1=cf32_t[0:32, 97:98], scalar2=None,
                                    op0=AO.add)
            nc.sync.dma_start(out=outt.ap(), in_=out_t[:])

    nc.compile()
    return nc


def kernel(**inputs):
    f32 = np.float32
    in_maps, NC, local = _host_prep(
        np.asarray(inputs["x_news"], f32),
        np.asarray(inputs["gat_n_ws"], f32), np.asarray(inputs["gat_n_as"], f32),
        np.asarray(inputs["gat_n_wd"], f32), np.asarray(inputs["gat_n_ad"], f32),
        np.asarray(inputs["gat_n_b"], f32),
        np.asarray(inputs["lin1_w"], f32), np.asarray(inputs["lin1_b"], f32),
        np.asarray(inputs["lin2_w"], f32), np.asarray(inputs["lin2_b"], f32),
        inputs["links_src"], inputs["links_dst"],
        inputs["n_id"], inputs["news_indices"])

    if NC not in _CACHE:
        _CACHE.clear()
        _CACHE[NC] = _build_program(NC)
    nc = _CACHE[NC]

    from concourse.bass_utils import run_bass_kernel_spmd
    res = run_bass_kernel_spmd(nc, in_maps, core_ids=list(range(NCORES)))

    out = np.empty((Q, 32), f32)
    for c in range(NCORES):
        out[c * SLOTS:(c + 1) * SLOTS] = res.results[c]["outt"].T
    return out


def _persistent_runner(nc, in_maps):
    """Build a reusable jitted 8-core executable with device-resident inputs.
    Returns (run_fn, fetch_fn) where run_fn() dispatches + blocks."""
    import jax
    import numpy as np_
    from jax.sharding import Mesh, PartitionSpec
    from jax.experimental.shard_map import shard_map
    import concourse.mybir as mybir
    from concourse.bass2jax import _bass_exec_p, install_neuronx_cc_hook

    install_neuronx_cc_hook()
    n_cores = len(in_maps)
    partition_name = nc.partition_id_tensor.name if nc.partition_id_tensor else None
    in_names, out_names, out_avals, zero_outs = [], [], [], []
    for alloc in nc.m.functions[0].allocations:
        if not isinstance(alloc, mybir.MemoryLocationSet):
            continue
        name = alloc.memorylocations[0].name
        if alloc.kind == "ExternalInput":
            if name != partition_name:
                in_names.append(name)
        elif alloc.kind == "ExternalOutput":
            shape = tuple(alloc.tensor_shape)
            dtype = mybir.dt.np(alloc.dtype)
            out_names.append(name)
            out_avals.append(jax.core.ShapedArray(shape, dtype))
            zero_outs.append(np_.zeros(shape, dtype))
    n_params = len(in_names)
    all_in = in_names + out_names
    if partition_name is not None:
        all_in.append(partition_name)

    def _body(*args):
        operands = list(args)
        if partition_name is not None:
            from concourse.bass2jax import partition_id_tensor
            operands.append(partition_id_tensor())
        return tuple(_bass_exec_p.bind(
            *operands, out_avals=tuple(out_avals), in_names=tuple(all_in),
            out_names=tuple(out_names), lowering_input_output_aliases=(),
            sim_require_finite=True, sim_require_nnan=True, nc=nc))

    devices = jax.devices()[:n_cores]
    mesh = Mesh(np_.asarray(devices), ("core",))
    nin = n_params + len(zero_outs)
    fn = jax.jit(shard_map(_body, mesh=mesh,
                           in_specs=(PartitionSpec("core"),) * nin,
                           out_specs=(PartitionSpec("core"),) * len(out_names),
                           check_rep=False))
    sh = jax.sharding.NamedSharding(mesh, PartitionSpec("core"))
    dev_in = [jax.device_put(
        np_.concatenate([np_.asarray(in_maps[c][n]) for c in range(n_cores)], axis=0), sh)
        for n in in_names]
    dev_zero = [jax.device_put(
        np_.zeros((n_cores * z.shape[0], *z.shape[1:]), z.dtype), sh) for z in zero_outs]

    state = {}

    def run_fn():
        out = fn(*dev_in, *dev_zero)
        jax.block_until_ready(out)
        state["out"] = out
        return out

    def fetch_fn():
        out = state["out"]
        return [{n: np_.asarray(out[i]).reshape(n_cores, *out_avals[i].shape)[c]
                 for i, n in enumerate(out_names)} for c in range(n_cores)]

    return run_fn, fetch_fn


def measure_hw_time(iters=12, **inputs):
    """Steady-state per-call wall time of the jitted executable, minus the
    dispatch baseline of a trivial program. Returns ns."""
    import time
    import concourse.bacc as bacc
    import concourse.mybir as mybir
    import concourse.tile as tile

    f32 = np.float32
    in_maps, NC, local = _host_prep(
        np.asarray(inputs["x_news"], f32),
        np.asarray(inputs["gat_n_ws"], f32), np.asarray(inputs["gat_n_as"], f32),
        np.asarray(inputs["gat_n_wd"], f32), np.asarray(inputs["gat_n_ad"], f32),
        np.asarray(inputs["gat_n_b"], f32),
        np.asarray(inputs["lin1_w"], f32), np.asarray(inputs["lin1_b"], f32),
        np.asarray(inputs["lin2_w"], f32), np.asarray(inputs["lin2_b"], f32),
        inputs["links_src"], inputs["links_dst"],
        inputs["n_id"], inputs["news_indices"])
    if NC not in _CACHE:
        _CACHE.clear()
        _CACHE[NC] = _build_program(NC)
    nc = _CACHE[NC]

    run_fn, _ = _persistent_runner(nc, in_maps)
    run_fn()  # compile + warm
    ts = []
    for _ in range(iters):
        t0 = time.perf_counter()
        run_fn()
        ts.append(time.perf_counter() - t0)
    t_kernel = min(ts)

    # trivial baseline program (same machinery, ~zero device work)
    f32m = mybir.dt.float32
    nb = bacc.Bacc("TRN2", target_bir_lowering=False, debug=False, num_devices=8)
    xi = nb.dram_tensor("xi", [128, 128], f32m, kind="ExternalInput")
    xo = nb.dram_tensor("xo", [128, 128], f32m, kind="ExternalOutput")
    with tile.TileContext(nb) as tc:
        with tc.tile_pool(name="p", bufs=1) as pool:
            t = pool.tile([128, 128], f32m)
            nb.sync.dma_start(out=t[:], in_=xi.ap())
            nb.sync.dma_start(out=xo.ap(), in_=t[:])
    nb.compile()
    base_maps = [dict(xi=np.zeros((128, 128), np.float32))] * 8
    brun, _ = _persistent_runner(nb, base_maps)
    brun()
    bs = []
    for _ in range(iters):
        t0 = time.perf_counter()
        brun()
        bs.append(time.perf_counter() - t0)
    t_base = min(bs)
    print(f"  [timing] kernel call: {t_kernel*1e3:.2f} ms, baseline: {t_base*1e3:.2f} ms")
    return max(t_kernel - t_base, 0.0) * 1e9
